# revision 1
# baseline (speedup 1.0000x reference)
"""Differential multi-head self-attention on 8 Trainium2 NeuronCores.

Sharding: core c handles batch b = c // 4 and heads {2*(c%4), 2*(c%4)+1}
(data parallel over batch, tensor parallel over heads). One SPMD Bass
program runs on all 8 cores; every per-core difference flows through the
input data. Each core emits per-head partial output projections
(o_h @ Wo_h, with rms_w and the (1 - lambda_init) factor folded into Wo
on the host); the host sums the partials per batch and adds bo.

Device math per (b, h):
  xT = x.T (PE transpose), QT/KT = W.T @ xT, V = x @ Wv   (f32r matmuls)
  per q tile of 256, k-chunk-major fused loop:
    St_half[k, q] = K_half @ Q_half.T  (f32r, causal tiles only)
    E_half = exp(St / sqrt(half)) * keep_mask   (ACT exp, bf16 output)
    O_half[q, d] += E_half[kc].T @ V[kc]        (bf16 matmuls, f32 PSUM)
    sums_half[q] += E_half[kc].T @ ones         (N=1 matmuls)
  O = O1/sums1 - lam * O2/sums2       (DVE per-partition scalars)
  r = exp(-0.5 * ln(eps + mean(O^2))) (ACT Square/Ln/Exp, single table set)
  out_h[s, e] = r[s] * (O.T @ Wo_h')[s, e]  (PE transpose + f32r matmul,
                                             r applied on the PSUM drain)
"""

import numpy as np
import ml_dtypes

import concourse.bass as bass
import concourse.mybir as mybir
import concourse.tile as tile
from concourse import bacc
from concourse.bass_utils import run_bass_kernel_spmd
from concourse.hw_specs import get_activation_tables
from concourse.masks import make_identity

B, S, E, H, D = 2, 2048, 512, 8, 512
HALF = D // 2
HLOC = 2            # heads per core
NCORES = 8
QT = 256            # q tile (free dim of score matmuls)
NQT = S // QT       # 8
KC = 128            # k chunk (partition dim of score tiles)
NKC = S // KC       # 16
NQC = QT // 128     # 2 q chunks per q tile
NDC = D // 128      # 4
NEC = E // 128      # 4
NSC = S // 128      # 16
KT_TILE = 512
SCALE = 1.0 / float(np.sqrt(HALF))
EPS = float(np.finfo(np.float32).eps)
LAMBDA_INIT = 0.8

f32 = mybir.dt.float32
f32r = mybir.dt.float32r
bf16 = mybir.dt.bfloat16
AF = mybir.ActivationFunctionType
ALU = mybir.AluOpType

SKIP, FULL = -1, -2


def _analyze_mask(mask):
    """Per (q-tile, k-chunk) block status: SKIP / FULL / keep-pattern index."""
    status = [[SKIP] * NKC for _ in range(NQT)]
    pats = []
    pat_idx = {}
    for t in range(NQT):
        for kc in range(NKC):
            blk = mask[t * QT:(t + 1) * QT, kc * KC:(kc + 1) * KC]  # [256 q, 128 k]
            if blk.all():
                status[t][kc] = SKIP
            elif not blk.any():
                status[t][kc] = FULL
            else:
                keep = (~blk).T.astype(np.float32)  # [128 k, 256 q]
                key = keep.tobytes()
                if key not in pat_idx:
                    pat_idx[key] = len(pats)
                    pats.append(keep)
                status[t][kc] = pat_idx[key]
    return status, pats


def _build(status, npat, repeat=1, unroll=1):  # noqa: C901
    nc = bacc.Bacc("TRN2", target_bir_lowering=False, debug=False)

    x_d = nc.dram_tensor("x", [S, E], f32, kind="ExternalInput")
    wq_d = nc.dram_tensor("wq", [HLOC, E, D], f32, kind="ExternalInput")
    wk_d = nc.dram_tensor("wk", [HLOC, E, D], f32, kind="ExternalInput")
    wv_d = nc.dram_tensor("wv", [HLOC, E, D], f32, kind="ExternalInput")
    wo_d = nc.dram_tensor("wo", [HLOC, D, E], f32, kind="ExternalInput")
    lamneg_d = nc.dram_tensor("lamneg", [HLOC, 128, 1], f32, kind="ExternalInput")
    keeps_d = nc.dram_tensor("keeps", [npat, 128, QT], bf16, kind="ExternalInput")
    out_d = nc.dram_tensor("out", [HLOC, S, E], f32, kind="ExternalOutput")
    iters_d = nc.dram_tensor("iters", [1, 1], f32, kind="ExternalOutput") if repeat > 1 else None

    act_sets = list(get_activation_tables(nc.m.arch).keys())
    nle_set = act_sets.index("natural_log_exp_and_others")

    with tile.TileContext(nc) as tc:
        with tc.tile_pool(name="cst", bufs=1) as cst, \
             tc.tile_pool(name="big", bufs=1) as big, \
             tc.tile_pool(name="epool", bufs=2) as epool, \
             tc.tile_pool(name="wts", bufs=1) as wts, \
             tc.tile_pool(name="qtp", bufs=2) as qtp, \
             tc.tile_pool(name="otp", bufs=2) as otp, \
             tc.tile_pool(name="scr", bufs=2) as scr, \
             tc.tile_pool(name="ps", bufs=3, space="PSUM") as ps, \
             tc.tile_pool(name="pso", bufs=4, space="PSUM") as pso, \
             tc.tile_pool(name="pss", bufs=1, space="PSUM") as pss:

            # One ACT table set covers Exp/Ln/Square: load it once up front.
            nc.scalar.add_instruction(mybir.InstLoadActFuncSet(
                name=nc.get_next_instruction_name(),
                ins=[], outs=[], act_func_set_id=nle_set))

            ident = cst.tile([128, 128], f32, tag="ident")
            make_identity(nc, ident[:])
            ones_bf = cst.tile([128, 1], bf16, tag="ones")
            nc.gpsimd.memset(ones_bf[:], 1.0)
            keeps_t = cst.tile([128, max(npat, 1), QT], bf16, tag="keeps")
            for i in range(npat):
                nc.sync.dma_start(keeps_t[:, i, :], keeps_d.ap()[i])
            lam_t = cst.tile([128, HLOC], f32, tag="lam")
            eps_t = cst.tile([128, 1], f32, tag="eps")
            nc.gpsimd.memset(eps_t[:], EPS)
            for h in range(HLOC):
                nc.sync.dma_start(lam_t[:, h:h + 1], lamneg_d.ap()[h])

            if repeat > 1:
                ctr = cst.tile([1, 1], f32, tag="ctr")
                nc.gpsimd.memset(ctr[:], 0.0)
            rep_ctx = tc.For_i(0, repeat, 1) if repeat > 1 else None
            if rep_ctx is not None:
                rep_ctx.__enter__()
                nc.vector.tensor_scalar_add(ctr[:], ctr[:], 1.0)

            def emit_body():
                # xT[e, s] from x[s, e] via PE transposes; x staged in 4 big DMAs
                xT = big.tile([128, NEC, S], f32r, tag="xT")
                for g in range(4):
                  xload = scr.tile([128, 4, E], f32, tag="xload")
                  nc.sync.dma_start(
                      xload[:],
                      x_d.ap()[g * 512:(g + 1) * 512, :].rearrange("(a p) e -> p a e", p=128))
                  for a in range(4):
                      st = g * 4 + a
                      for ec in range(NEC):
                          tp = ps.tile([128, 128], f32, tag="mmps")
                          nc.tensor.transpose(tp[:], xload[:, a, ec * 128:(ec + 1) * 128], ident[:])
                          nc.vector.tensor_copy(xT[:, ec, st * 128:(st + 1) * 128], tp[:])

                for h in range(HLOC):
                  wq_t = wts.tile([128, NEC, D], f32r, tag="wq")
                  wk_t = wts.tile([128, NEC, D], f32r, tag="wk")
                  wv_t = wts.tile([128, NEC, D], f32r, tag="wv")
                  wo_t = wts.tile([128, NDC, E], f32r, tag="wo")
                  for ec in range(NEC):
                      nc.sync.dma_start(wq_t[:, ec, :], wq_d.ap()[h, ec * 128:(ec + 1) * 128, :].bitcast(f32r))
                      nc.sync.dma_start(wk_t[:, ec, :], wk_d.ap()[h, ec * 128:(ec + 1) * 128, :].bitcast(f32r))
                      nc.sync.dma_start(wv_t[:, ec, :], wv_d.ap()[h, ec * 128:(ec + 1) * 128, :].bitcast(f32r))
                  for dc in range(NDC):
                      nc.sync.dma_start(wo_t[:, dc, :], wo_d.ap()[h, dc * 128:(dc + 1) * 128, :].bitcast(f32r))

                  # KT[d, k] = Wk.T @ xT
                  KT = big.tile([128, NDC, S], f32r, tag="KT")
                  for dc in range(NDC):
                      for kt in range(S // KT_TILE):
                          kps = ps.tile([128, KT_TILE], f32, tag="mmps")
                          for ec in range(NEC):
                              nc.tensor.matmul(
                                  kps[:],
                                  wk_t[:, ec, dc * 128:(dc + 1) * 128],
                                  xT[:, ec, kt * KT_TILE:(kt + 1) * KT_TILE],
                                  start=(ec == 0), stop=(ec == NEC - 1))
                          nc.vector.tensor_copy(KT[:, dc, kt * KT_TILE:(kt + 1) * KT_TILE], kps[:])

                  # V[s, d] = x @ Wv  (bf16)
                  V = big.tile([128, NKC, D], bf16, tag="V")
                  for sc in range(NSC):
                      vps = ps.tile([128, D], f32, tag="mmps")
                      for ec in range(NEC):
                          nc.tensor.matmul(
                              vps[:],
                              xT[:, ec, sc * 128:(sc + 1) * 128],
                              wv_t[:, ec, :],
                              start=(ec == 0), stop=(ec == NEC - 1))
                      nc.scalar.activation(V[:, sc, :], vps[:], AF.Copy)

                  def emit_tail(st_):
                      # transposes + outproj of a finished q tile (PE work that
                      # depends on the DVE/ACT combine chain); emitted after the
                      # next tile's head so PE never stalls on that chain.
                      q0_, osbs_, rr_ = st_
                      for qc in range(NQC):
                          qq = q0_ + qc * 128
                          ot_t = otp.tile([128, NDC, 128], f32r, tag="ot", name=f"ot{qc}")
                          for dc in range(NDC):
                              tp = ps.tile([128, 128], f32, tag="mmps", name="tp2")
                              nc.tensor.transpose(tp[:], osbs_[qc][:, dc * 128:(dc + 1) * 128], ident[:])
                              nc.vector.tensor_copy(ot_t[:, dc, :], tp[:])
                          out_ps = ps.tile([128, E], f32, tag="mmps", name="out_ps")
                          for dc in range(NDC):
                              nc.tensor.matmul(
                                  out_ps[:], ot_t[:, dc, :], wo_t[:, dc, :],
                                  start=(dc == 0), stop=(dc == NDC - 1))
                          out_sb = scr.tile([128, E], f32, tag="outsb")
                          nc.vector.tensor_scalar_mul(out_sb[:], out_ps[:], rr_[:, qc:qc + 1])
                          nc.sync.dma_start(out_d.ap()[h, qq:qq + 128, :], out_sb[:])

                  pending = None
                  for t in range(NQT):
                      q0 = t * QT
                      kcs = [kc for kc in range(NKC) if status[t][kc] != SKIP]
                      nk = len(kcs)

                      # QT[d, q] for this q tile
                      qt_t = qtp.tile([128, NDC, QT], f32r, tag="qt")
                      for dc in range(NDC):
                          qps = ps.tile([128, QT], f32, tag="mmps")
                          for ec in range(NEC):
                              nc.tensor.matmul(
                                  qps[:],
                                  wq_t[:, ec, dc * 128:(dc + 1) * 128],
                                  xT[:, ec, q0:q0 + QT],
                                  start=(ec == 0), stop=(ec == NEC - 1))
                          nc.scalar.activation(qt_t[:, dc, :], qps[:], AF.Copy)

                      E1 = epool.tile([128, NKC, QT], bf16, tag="E1")
                      E2 = epool.tile([128, NKC, QT], bf16, tag="E2")
                      sums_ps = pss.tile([128, 2 * NQC], f32, tag="sums")
                      o_ps = [[None] * NQC, [None] * NQC]
                      for half in (0, 1):
                          for qc in range(NQC):
                              o_ps[half][qc] = pso.tile([128, D], f32, tag="ops", name=f"ops{half}{qc}")

                      # fused k-chunk-major: scores -> exp(+mask); AV + sums run
                      # one k chunk behind so PE never waits on the exp latency
                      def emit_av(i, kc):
                          for half in (0, 1):
                              Et = E1 if half == 0 else E2
                              for qc in range(NQC):
                                  lhsT = Et[:, kc, qc * 128:(qc + 1) * 128]
                                  nc.tensor.matmul(
                                      o_ps[half][qc][:], lhsT, V[:, kc, :],
                                      start=(i == 0), stop=(i == nk - 1))
                                  nc.tensor.matmul(
                                      sums_ps[:, half * NQC + qc:half * NQC + qc + 1],
                                      lhsT, ones_bf[:],
                                      start=(i == 0 and half == 0 and qc == 0),
                                      stop=(i == nk - 1 and half == 1 and qc == NQC - 1))

                      for i, kc in enumerate(kcs):
                          for half in (0, 1):
                              sps = ps.tile([128, QT], f32, tag="mmps")
                              for j in (0, 1):
                                  dc = half * 2 + j
                                  nc.tensor.matmul(
                                      sps[:],
                                      KT[:, dc, kc * 128:(kc + 1) * 128],
                                      qt_t[:, dc, :],
                                      start=(j == 0), stop=(j == 1))
                              Et = E1 if half == 0 else E2
                              nc.scalar.activation(Et[:, kc, :], sps[:], AF.Exp, scale=SCALE)
                              pat = status[t][kc]
                              if pat >= 0:
                                  nc.vector.tensor_tensor(
                                      out=Et[:, kc, :], in0=Et[:, kc, :],
                                      in1=keeps_t[:, pat, :], op=ALU.mult)
                          if i > 0:
                              emit_av(i - 1, kcs[i - 1])
                      emit_av(nk - 1, kcs[nk - 1])

                      if pending is not None:
                          emit_tail(pending)
                          pending = None

                      # combine + rms per q chunk (DVE/ACT; overlaps next head)
                      rec = scr.tile([128, 2 * NQC], f32, tag="rec")
                      nc.vector.reciprocal(rec[:], sums_ps[:])
                      nc.vector.tensor_tensor(
                          out=rec[:, NQC:2 * NQC],
                          in0=rec[:, NQC:2 * NQC],
                          in1=lam_t[:, h:h + 1].to_broadcast([128, NQC]),
                          op=ALU.mult)
                      ms = scr.tile([128, NQC], f32, tag="ms")
                      osbs = []
                      for qc in range(NQC):
                          osb = scr.tile([128, D], f32, tag=f"osb{qc}")
                          nc.vector.tensor_scalar_mul(osb[:], o_ps[0][qc][:], rec[:, qc:qc + 1])
                          osb2 = scr.tile([128, D], f32, tag=f"osb2{qc}")
                          nc.vector.scalar_tensor_tensor(
                              out=osb2[:], in0=o_ps[1][qc][:],
                              scalar=rec[:, NQC + qc:NQC + qc + 1],
                              in1=osb[:], op0=ALU.mult, op1=ALU.add)
                          osq = scr.tile([128, D], f32, tag="osq")
                          nc.scalar.activation(
                              osq[:], osb2[:], AF.Square,
                              scale=float(1.0 / np.sqrt(D)), accum_out=ms[:, qc:qc + 1])
                          osbs.append(osb2)
                      lnm = scr.tile([128, NQC], f32, tag="lnm")
                      nc.scalar.activation(lnm[:], ms[:], AF.Ln, bias=eps_t[:])
                      rr = scr.tile([128, NQC], f32, tag="rr")
                      nc.scalar.activation(rr[:], lnm[:], AF.Exp, scale=-0.5)
                      pending = (q0, osbs, rr)
                  if pending is not None:
                      emit_tail(pending)
                      pending = None

            for _u in range(unroll):
                emit_body()

            if rep_ctx is not None:
                rep_ctx.__exit__(None, None, None)
                nc.sync.dma_start(iters_d.ap()[:], ctr[:])

    nc.compile()
    return nc


_CACHE = {}


def _get_program(mask, repeat=1, unroll=1):
    key = (mask.tobytes(), repeat, unroll)
    if key not in _CACHE:
        status, pats = _analyze_mask(mask)
        nc = _build(status, len(pats), repeat=repeat, unroll=unroll)
        _CACHE[key] = (nc, pats)
    return _CACHE[key]


def make_in_maps(x, mask, Wq, bq, Wk, bk, Wv, bv, lq1, lk1, lq2, lk2,
                 lam_init_p, rms_w, Wo, bo, repeat=1, unroll=1):
    x = np.asarray(x, np.float32)
    mask = np.asarray(mask, bool)
    Wq = np.asarray(Wq, np.float32)
    Wk = np.asarray(Wk, np.float32)
    Wv = np.asarray(Wv, np.float32)
    Wo = np.asarray(Wo, np.float32)
    for b_ in (bq, bk, bv):
        assert np.abs(np.asarray(b_)).max() == 0.0, "nonzero qkv bias unsupported"
    lam = (np.exp((np.asarray(lq1, np.float32) * np.asarray(lk1, np.float32)).sum(-1))
           - np.exp((np.asarray(lq2, np.float32) * np.asarray(lk2, np.float32)).sum(-1))
           + np.asarray(lam_init_p, np.float32))  # [H]
    woF = Wo.reshape(H, D, E) * ((1.0 - LAMBDA_INIT) * np.asarray(rms_w, np.float32))[:, :, None]

    nc, pats = _get_program(mask, repeat=repeat, unroll=unroll)
    if pats:
        keeps = np.stack(pats).astype(ml_dtypes.bfloat16)
    else:
        keeps = np.zeros((1, 128, QT), ml_dtypes.bfloat16)

    in_maps = []
    for c in range(NCORES):
        b = c // 4
        h0 = HLOC * (c % 4)
        lamneg = np.repeat((-lam[h0:h0 + HLOC]).astype(np.float32)[:, None, None], 128, axis=1)
        in_maps.append({
            "x": np.ascontiguousarray(x[b]),
            "wq": np.ascontiguousarray(Wq[h0:h0 + HLOC]),
            "wk": np.ascontiguousarray(Wk[h0:h0 + HLOC]),
            "wv": np.ascontiguousarray(Wv[h0:h0 + HLOC]),
            "wo": np.ascontiguousarray(woF[h0:h0 + HLOC]),
            "lamneg": np.ascontiguousarray(lamneg),
            "keeps": keeps,
        })
    return nc, in_maps


def gather(results, bo):
    out = np.zeros((B, S, E), np.float32)
    for c in range(NCORES):
        out[c // 4] += results[c]["out"].sum(axis=0)
    out += np.asarray(bo, np.float32)[None, None, :]
    return out


def kernel(**inputs):
    nc, in_maps = make_in_maps(**inputs)
    res = run_bass_kernel_spmd(nc, in_maps, core_ids=list(range(NCORES)))
    return gather(res.results, inputs["bo"])



# revision 9
# speedup vs baseline: 1.5553x; 1.5553x over previous
"""Differential multi-head self-attention on 8 Trainium2 NeuronCores.

Sharding: core c handles batch b = c // 4 and heads {2*(c%4), 2*(c%4)+1}
(data parallel over batch, tensor parallel over heads). One SPMD Bass
program runs on all 8 cores; every per-core difference flows through the
input data. Each core emits per-head partial output projections
(o_h @ Wo_h, with rms_w and the (1 - lambda_init) factor folded into Wo
on the host); the host sums the partials per batch and adds bo.

Device math per (b, h), scores kept in [q_partition, k_free] orientation:
  xT = x.T (PE transpose) -> xbf (bf16) + x8 (fp8e4, for QK projections)
  QT/KT = (16*W8).T @ x8 via fp8 DoubleRow matmuls -> bf16 [d, s]
  V = x @ Wv (bf16)
  per q chunk of 128 rows:
    S_half[q, k] = Q_half.T K_half  (bf16, Q chunk stationary, k streamed)
    masked blocks get -1e8 added in PSUM (DVE), so exp -> 0
    E_half = exp(S / (sqrt(half)*256)), row sums via ACT accum_out
    W = E1 - (lam*s1/s2) * E2      (single DVE pass, per-partition scalar;
                                    softmax denominators cancel in RMSNorm)
    u[q, d] = W @ V  (one bf16 AV chain via PE-transposed W blocks)
    rr = exp(-0.5*ln(eps + mean(u^2)))  (ACT Square/Ln/Exp)
    out_h[q, e] = rr[q] * (u.T @ Wo_h')[q, e]
"""

import numpy as np
import ml_dtypes

import concourse.bass as bass
import concourse.mybir as mybir
import concourse.tile as tile
from concourse import bacc
from concourse.bass_utils import run_bass_kernel_spmd
from concourse.hw_specs import get_activation_tables
from concourse.masks import make_identity

B, S, E, H, D = 2, 2048, 512, 8, 512
HALF = D // 2
HLOC = 2            # heads per core
NCORES = 8
NEC = E // 128      # 4 e chunks
NDC = D // 128      # 4 d chunks
NQC = S // 128      # 16 q chunks
NKC = S // 128      # 16 k chunks (128 wide)
STRIP = 512         # k strip width for score matmuls / exp
SCALE = 1.0 / float(np.sqrt(HALF))
SCL = SCALE
EPS = float(np.finfo(np.float32).eps)
LAMBDA_INIT = 0.8
MASK_BIAS = -1.0e8

f32 = mybir.dt.float32
bf16 = mybir.dt.bfloat16
AF = mybir.ActivationFunctionType
ALU = mybir.AluOpType


def _analyze_mask(mask):
    """Per 128-q-chunk: (kept k extent, [(kc, pattern_idx), ...]).

    Requires kept k blocks to form a prefix per q chunk (causal family).
    Patterns are additive f32 bias blocks: MASK_BIAS where masked.
    """
    info = []
    pats = []
    pat_idx = {}
    for qc in range(NQC):
        rows = mask[qc * 128:(qc + 1) * 128]
        last = -1
        blocks = []
        for kc in range(NKC):
            blk = rows[:, kc * 128:(kc + 1) * 128]
            if blk.all():
                continue
            assert last == kc - 1, "kept k blocks must be a prefix per q chunk"
            last = kc
            if blk.any():
                key = blk.tobytes()
                if key not in pat_idx:
                    pat_idx[key] = len(pats)
                    pats.append(np.where(blk, MASK_BIAS, 0.0).astype(np.float32))
                blocks.append((kc, pat_idx[key]))
        assert last >= 0, "q chunk with all k masked unsupported"
        info.append(((last + 1) * 128, blocks))
    return info, pats


def _strips(kext):
    out = []
    k0 = 0
    while k0 < kext:
        out.append((k0, min(STRIP, kext - k0)))
        k0 += STRIP
    return out


def _build(info, npat, repeat=1, unroll=1):  # noqa: C901
    nc = bacc.Bacc("TRN2", target_bir_lowering=False, debug=False)

    x_d = nc.dram_tensor("x", [S, E], f32, kind="ExternalInput")
    wq_d = nc.dram_tensor("wq", [HLOC, E, D], bf16, kind="ExternalInput")
    wk_d = nc.dram_tensor("wk", [HLOC, E, D], bf16, kind="ExternalInput")
    wv_d = nc.dram_tensor("wv", [HLOC, E, D], bf16, kind="ExternalInput")
    wo_d = nc.dram_tensor("wo", [HLOC, D, E], bf16, kind="ExternalInput")
    lamneg_d = nc.dram_tensor("lamneg", [HLOC, 128, 1], f32, kind="ExternalInput")
    biases_d = nc.dram_tensor("biases", [npat, 128, 128], f32, kind="ExternalInput")
    out_d = nc.dram_tensor("out", [HLOC, S, E], f32, kind="ExternalOutput")
    iters_d = nc.dram_tensor("iters", [1, 1], f32, kind="ExternalOutput") if repeat > 1 else None

    act_sets = list(get_activation_tables(nc.m.arch).keys())
    nle_set = act_sets.index("natural_log_exp_and_others")

    with tile.TileContext(nc) as tc:
        with tc.tile_pool(name="cst", bufs=1) as cst, \
             tc.tile_pool(name="big", bufs=1) as big, \
             tc.tile_pool(name="wts", bufs=1) as wts, \
             tc.tile_pool(name="epool", bufs=2) as epool, \
             tc.tile_pool(name="wpool", bufs=2) as wpool, \
             tc.tile_pool(name="upool", bufs=3) as upool, \
             tc.tile_pool(name="scr", bufs=2) as scr, \
             tc.tile_pool(name="psS", bufs=3, space="PSUM") as psS, \
             tc.tile_pool(name="pacc", bufs=3, space="PSUM") as pacc, \
             tc.tile_pool(name="ptr", bufs=2, space="PSUM") as ptr:

            nc.scalar.add_instruction(mybir.InstLoadActFuncSet(
                name=nc.get_next_instruction_name(),
                ins=[], outs=[], act_func_set_id=nle_set))

            ident = cst.tile([128, 128], f32, tag="ident")
            make_identity(nc, ident[:])
            ident_bf = cst.tile([128, 128], bf16, tag="ident_bf")
            nc.vector.tensor_copy(ident_bf[:], ident[:])
            lam_t = cst.tile([128, HLOC], f32, tag="lam")
            eps_t = cst.tile([128, 1], f32, tag="eps")
            nc.gpsimd.memset(eps_t[:], EPS)
            for h in range(HLOC):
                nc.sync.dma_start(lam_t[:, h:h + 1], lamneg_d.ap()[h])
            bias_t = cst.tile([128, max(npat, 1), 128], f32, tag="bias")
            for i in range(npat):
                nc.sync.dma_start(bias_t[:, i, :], biases_d.ap()[i])

            if repeat > 1:
                ctr = cst.tile([1, 1], f32, tag="ctr")
                nc.gpsimd.memset(ctr[:], 0.0)
            rep_ctx = tc.For_i(0, repeat, 1) if repeat > 1 else None
            if rep_ctx is not None:
                rep_ctx.__enter__()
                nc.vector.tensor_scalar_add(ctr[:], ctr[:], 1.0)

            def emit_body():
                # x transposed to bf16 [e, s]
                xbf = big.tile([128, NEC, S], bf16, tag="xbf")
                for g in range(4):
                    xload = scr.tile([128, 4, E], f32, tag="xload")
                    nc.sync.dma_start(
                        xload[:],
                        x_d.ap()[g * 512:(g + 1) * 512, :].rearrange("(a p) e -> p a e", p=128))
                    for ec in range(NEC):
                        tp = pacc.tile([128, 512], f32, tag="acc", name="xtp")
                        for a in range(4):
                            nc.tensor.transpose(
                                tp[:, a * 128:(a + 1) * 128],
                                xload[:, a, ec * 128:(ec + 1) * 128], ident[:])
                        nc.vector.tensor_copy(xbf[:, ec, g * 512:(g + 1) * 512], tp[:])

                for h in range(HLOC):
                    wq_t = wts.tile([128, NEC, D], bf16, tag="wq")
                    wk_t = wts.tile([128, NEC, D], bf16, tag="wk")
                    wv_t = wts.tile([128, NEC, D], bf16, tag="wv")
                    wo_t = wts.tile([128, NDC, E], bf16, tag="wo")
                    for ec in range(NEC):
                        nc.sync.dma_start(wq_t[:, ec, :], wq_d.ap()[h, ec * 128:(ec + 1) * 128, :])
                        nc.sync.dma_start(wk_t[:, ec, :], wk_d.ap()[h, ec * 128:(ec + 1) * 128, :])
                        nc.sync.dma_start(wv_t[:, ec, :], wv_d.ap()[h, ec * 128:(ec + 1) * 128, :])
                    for dc in range(NDC):
                        nc.sync.dma_start(wo_t[:, dc, :], wo_d.ap()[h, dc * 128:(dc + 1) * 128, :])

                    # QT/KT [d, s] bf16
                    qtb = big.tile([128, NDC, S], bf16, tag="qtb")
                    ktb = big.tile([128, NDC, S], bf16, tag="ktb")
                    for wt, dst, dve in ((wk_t, ktb, True), (wq_t, qtb, False)):
                        for dc in range(NDC):
                            for s4 in range(S // 512):
                                kps = psS.tile([128, 512], f32, tag="sps", name="kps")
                                for ec in range(NEC):
                                    nc.tensor.matmul(
                                        kps[:],
                                        wt[:, ec, dc * 128:(dc + 1) * 128],
                                        xbf[:, ec, s4 * 512:(s4 + 1) * 512],
                                        start=(ec == 0), stop=(ec == NEC - 1))
                                if dve:
                                    nc.vector.tensor_copy(dst[:, dc, s4 * 512:(s4 + 1) * 512], kps[:])
                                else:
                                    nc.scalar.activation(dst[:, dc, s4 * 512:(s4 + 1) * 512], kps[:], AF.Copy)

                    # V[s, d] bf16
                    V = big.tile([128, NKC, D], bf16, tag="V")
                    for sc in range(NKC):
                        vps = psS.tile([128, D], f32, tag="sps", name="vps")
                        for ec in range(NEC):
                            nc.tensor.matmul(
                                vps[:],
                                xbf[:, ec, sc * 128:(sc + 1) * 128],
                                wv_t[:, ec, :],
                                start=(ec == 0), stop=(ec == NEC - 1))
                        nc.scalar.activation(V[:, sc, :], vps[:], AF.Copy)

                    ms = scr.tile([128, NQC], f32, tag="ms")
                    lnv = scr.tile([128, NQC], f32, tag="lnv")
                    rr = scr.tile([128, NQC], f32, tag="rr")
                    state = {}

                    def emit_S(qc):
                        kext, patblocks = info[qc]
                        strips = _strips(kext)
                        ns = len(strips)
                        # E12: per strip, [E1(512) | E2(512)]
                        E12 = epool.tile([128, 4, 2 * STRIP], bf16, tag="E12", name="E12")
                        sumsp = scr.tile([128, 2, 4], f32, tag="sumsp", name="sumsp")
                        for si, (k0, w) in enumerate(strips):
                            for half in (0, 1):
                                sps = psS.tile([128, 512], f32, tag="sps", name="sps")
                                for j in (0, 1):
                                    dc = half * 2 + j
                                    nc.tensor.matmul(
                                        sps[:, 0:w],
                                        qtb[:, dc, qc * 128:(qc + 1) * 128],
                                        ktb[:, dc, k0:k0 + w],
                                        start=(j == 0), stop=(j == 1))
                                for (kc, pat) in patblocks:
                                    if k0 <= kc * 128 < k0 + w:
                                        off = kc * 128 - k0
                                        nc.vector.tensor_tensor(
                                            out=sps[:, off:off + 128], in0=sps[:, off:off + 128],
                                            in1=bias_t[:, pat, :], op=ALU.add)
                                nc.scalar.activation(
                                    E12[:, si, half * STRIP:half * STRIP + w], sps[:, 0:w],
                                    AF.Exp, scale=SCL, accum_out=sumsp[:, half, si:si + 1])
                        # row sums -> ratio rn = -lam * s1 / s2
                        ssum = scr.tile([128, 2], f32, tag="ssum", name="ssum")
                        nc.vector.tensor_reduce(
                            ssum[:], sumsp[:, :, 0:ns], axis=mybir.AxisListType.X, op=ALU.add)
                        rn = scr.tile([128, 1], f32, tag="rn", name="rn")
                        nc.vector.reciprocal(rn[:], ssum[:, 1:2])
                        nc.vector.tensor_tensor(out=rn[:], in0=rn[:], in1=ssum[:, 0:1], op=ALU.mult)
                        nc.vector.tensor_tensor(out=rn[:], in0=rn[:], in1=lam_t[:, h:h + 1], op=ALU.mult)
                        # W = E1 + rn*E2  (rn = -lam*s1/s2)
                        W = wpool.tile([128, NKC // 4, STRIP], bf16, tag="W", name="W")
                        nf, rem = kext // STRIP, kext % STRIP
                        if nf:
                            nc.vector.scalar_tensor_tensor(
                                out=W[:, 0:nf, :], in0=E12[:, 0:nf, STRIP:2 * STRIP],
                                scalar=rn[:], in1=E12[:, 0:nf, 0:STRIP],
                                op0=ALU.mult, op1=ALU.add)
                        if rem:
                            nc.vector.scalar_tensor_tensor(
                                out=W[:, nf, 0:rem], in0=E12[:, nf, STRIP:STRIP + rem],
                                scalar=rn[:], in1=E12[:, nf, 0:rem],
                                op0=ALU.mult, op1=ALU.add)
                        state[qc] = W

                    def emit_T(qc):
                        kext, _ = info[qc]
                        nb = kext // 128
                        W = state.pop(qc)
                        # transpose W blocks -> WT [k, q] (AV lhsT), 8 per bank
                        WT = wpool.tile([128, NKC, 128], bf16, tag="WT", name="WT")
                        for g0 in range(0, nb, 8):
                            gn = min(8, nb - g0)
                            wtp = ptr.tile([128, 1024], bf16, tag="tp", name="wtp")
                            for j in range(gn):
                                kc = g0 + j
                                nc.tensor.transpose(
                                    wtp[:, j * 128:(j + 1) * 128],
                                    W[:, kc // 4, (kc % 4) * 128:(kc % 4) * 128 + 128],
                                    ident_bf[:])
                            nc.vector.tensor_copy(
                                WT[:, g0:g0 + gn, :], wtp[:, 0:gn * 128])
                        # u[q, d] = W @ V
                        ops = pacc.tile([128, D], f32, tag="acc", name="ops")
                        for kc in range(nb):
                            nc.tensor.matmul(
                                ops[:], WT[:, kc, :], V[:, kc, :],
                                start=(kc == 0), stop=(kc == nb - 1))
                        sqv = scr.tile([128, D], f32, tag="sqv", name="sqv")
                        nc.scalar.activation(
                            sqv[:], ops[:], AF.Square,
                            scale=float(1.0 / np.sqrt(D)), accum_out=ms[:, qc:qc + 1])
                        if qc % 2 == 1 or qc == NQC - 1:
                            q0 = qc - 1 if qc % 2 == 1 else qc
                            nc.scalar.activation(lnv[:, q0:qc + 1], ms[:, q0:qc + 1], AF.Ln, bias=eps_t[:])
                            nc.scalar.activation(rr[:, q0:qc + 1], lnv[:, q0:qc + 1], AF.Exp, scale=-0.5)
                        u = upool.tile([128, D], bf16, tag="u", name="u")
                        nc.vector.tensor_copy(u[:], ops[:])
                        state[(qc, "u")] = u

                    def emit_O(qc):
                        u = state.pop((qc, "u"))
                        utp = ptr.tile([128, 1024], bf16, tag="tp", name="utp")
                        for dc in range(NDC):
                            nc.tensor.transpose(
                                utp[:, dc * 128:(dc + 1) * 128],
                                u[:, dc * 128:(dc + 1) * 128], ident_bf[:])
                        uT = upool.tile([128, NDC, 128], bf16, tag="uT", name="uT")
                        nc.vector.tensor_copy(uT[:], utp[:, 0:512])
                        outp = pacc.tile([128, E], f32, tag="acc", name="outp")
                        for dc in range(NDC):
                            nc.tensor.matmul(
                                outp[:], uT[:, dc, :], wo_t[:, dc, :],
                                start=(dc == 0), stop=(dc == NDC - 1))
                        out_sb = scr.tile([128, E], f32, tag="outsb", name="outsb")
                        nc.vector.tensor_scalar_mul(out_sb[:], outp[:], rr[:, qc:qc + 1])
                        nc.sync.dma_start(out_d.ap()[h, qc * 128:(qc + 1) * 128, :], out_sb[:])

                    for qc in range(NQC):
                        emit_S(qc)
                        if qc >= 1:
                            emit_T(qc - 1)
                        if qc >= 2:
                            emit_O(qc - 2)
                    emit_T(NQC - 1)
                    emit_O(NQC - 2)
                    emit_O(NQC - 1)

            for _u in range(unroll):
                emit_body()

            if rep_ctx is not None:
                rep_ctx.__exit__(None, None, None)
                nc.sync.dma_start(iters_d.ap()[:], ctr[:])

    nc.compile()
    return nc


_CACHE = {}


def _get_program(mask, repeat=1, unroll=1):
    key = (mask.tobytes(), repeat, unroll)
    if key not in _CACHE:
        info, pats = _analyze_mask(mask)
        nc = _build(info, len(pats), repeat=repeat, unroll=unroll)
        _CACHE[key] = (nc, pats)
    return _CACHE[key]


def make_in_maps(x, mask, Wq, bq, Wk, bk, Wv, bv, lq1, lk1, lq2, lk2,
                 lam_init_p, rms_w, Wo, bo, repeat=1, unroll=1):
    x = np.asarray(x, np.float32)
    mask = np.asarray(mask, bool)
    Wq = np.asarray(Wq, np.float32)
    Wk = np.asarray(Wk, np.float32)
    Wv = np.asarray(Wv, np.float32)
    Wo = np.asarray(Wo, np.float32)
    for b_ in (bq, bk, bv):
        assert np.abs(np.asarray(b_)).max() == 0.0, "nonzero qkv bias unsupported"
    lam = (np.exp((np.asarray(lq1, np.float32) * np.asarray(lk1, np.float32)).sum(-1))
           - np.exp((np.asarray(lq2, np.float32) * np.asarray(lk2, np.float32)).sum(-1))
           + np.asarray(lam_init_p, np.float32))  # [H]
    woF = Wo.reshape(H, D, E) * ((1.0 - LAMBDA_INIT) * np.asarray(rms_w, np.float32))[:, :, None]

    nc, pats = _get_program(mask, repeat=repeat, unroll=unroll)
    bfnp = ml_dtypes.bfloat16
    if pats:
        biases = np.stack(pats).astype(np.float32)
    else:
        biases = np.zeros((1, 128, 128), np.float32)
    wqb = Wq.astype(bfnp)
    wkb = Wk.astype(bfnp)
    wvb = Wv.astype(bfnp)
    wob = woF.astype(bfnp)

    in_maps = []
    for c in range(NCORES):
        b = c // 4
        h0 = HLOC * (c % 4)
        lamneg = np.repeat((-lam[h0:h0 + HLOC]).astype(np.float32)[:, None, None], 128, axis=1)
        in_maps.append({
            "x": np.ascontiguousarray(x[b]),
            "wq": np.ascontiguousarray(wqb[h0:h0 + HLOC]),
            "wk": np.ascontiguousarray(wkb[h0:h0 + HLOC]),
            "wv": np.ascontiguousarray(wvb[h0:h0 + HLOC]),
            "wo": np.ascontiguousarray(wob[h0:h0 + HLOC]),
            "lamneg": np.ascontiguousarray(lamneg),
            "biases": biases,
        })
    return nc, in_maps


def gather(results, bo):
    out = np.zeros((B, S, E), np.float32)
    for c in range(NCORES):
        out[c // 4] += results[c]["out"].sum(axis=0)
    out += np.asarray(bo, np.float32)[None, None, :]
    return out


def kernel(**inputs):
    nc, in_maps = make_in_maps(**inputs)
    res = run_bass_kernel_spmd(nc, in_maps, core_ids=list(range(NCORES)))
    return gather(res.results, inputs["bo"])
